# revision 16
# baseline (speedup 1.0000x reference)
"""Colight GNN message-passing kernel for 8x TRN2 NeuronCores.

Sharding: data-parallel over batch (16 batches -> 2 per core), weights replicated.
Per core, per batch:
  - MLP: x = relu(relu(agent@W1+b1)@W2+b2), computed transposed (xT [32, A]).
  - adj [A,N,A] is DMA'd once with inline f32->fp16 cast, PE-transposed into
    adjT [a, (n,i)] fp16 cached in SBUF, reused by both attention layers.
  - nbrT = (adj @ x)^T via matmul(lhsT=x_nat, rhs=adjT) accumulating over a-tiles.
  - Projections nr/nh/ah via stationary weights (lhsT=w [32,128], rhs=activationT).
  - scores via DVE product P = ahT*nrT + selector matmul (heads interleaved c=dv*8+h).
  - softmax over agents = free-dim softmax on scores40 [40(n,h), A].
  - out einsum via attB (selector broadcast) * nhT, accumulated over n in PSUM.
  - Layer 1 value path only computed for agent 0 (only agent 0 feeds q).
"""
import sys
sys.path.insert(0, '/opt/trn_rl_repo')

import numpy as np

import concourse.bass as bass
import concourse.mybir as mybir
from concourse import bacc
from concourse.tile import TileContext
from concourse.bass_utils import run_bass_kernel_spmd

# dims
B, A, N = 16, 859, 5
D = 32
H1 = 128
DV, NV, DOUT, ACT = 16, 8, 32, 8
C = DV * NV                    # 128, c = dv*8 + h
NB = 2                         # batches per core
NCORES = 8
J = N * A                      # 4295 flattened (n, i)
AT = (A + 127) // 128          # 7 a-tiles
ATAIL = A - (AT - 1) * 128     # 91
F16 = mybir.dt.float16
F32 = mybir.dt.float32
AF = mybir.ActivationFunctionType
AX = mybir.AxisListType


def _chunks(total, size):
    out = []
    o = 0
    while o < total:
        out.append((o, min(size, total - o)))
        o += size
    return out


J_CHUNKS = _chunks(J, 512)          # 9 chunks for nbr/nr/nh over (n,i)
A_CHUNKS = _chunks(A, 512)          # 2 chunks over agents
JDMA_CHUNKS = [(0, 512)] + [(512 + o, c) for (o, c) in _chunks(J - 512, 1024)]


def asize(t):
    return 128 if t < AT - 1 else ATAIL


def build_nc():
    nc = bacc.Bacc("TRN2", target_bir_lowering=False)

    # ---- dram inputs ----
    agentT_d = nc.dram_tensor("agentT", [NB, D, A], F16, kind="ExternalInput")
    adjT_d = nc.dram_tensor("adjT", [NB, A, J], F16, kind="ExternalInput")
    w1_d = nc.dram_tensor("w1", [D, H1], F16, kind="ExternalInput")
    w2_d = nc.dram_tensor("w2", [H1, D], F16, kind="ExternalInput")
    b1_d = nc.dram_tensor("b1", [H1, 1], F32, kind="ExternalInput")
    b2_d = nc.dram_tensor("b2", [D, 1], F32, kind="ExternalInput")
    finw_d = nc.dram_tensor("finw", [DOUT, ACT], F16, kind="ExternalInput")
    finb_d = nc.dram_tensor("finb", [ACT, 1], F32, kind="ExternalInput")
    lw = {}
    for l in range(2):
        for nm, shp in (("ah_w", [D, C]), ("nr_w", [D, C]), ("nh_w", [D, C]),
                        ("o_w", [DV, DOUT])):
            lw[(l, nm)] = nc.dram_tensor(f"l{l}_{nm}", shp, F16, kind="ExternalInput")
        for nm, p in (("ah_b", C), ("nr_b", C), ("nh_b", C), ("o_b", DOUT)):
            lw[(l, nm)] = nc.dram_tensor(f"l{l}_{nm}", [p, 1], F32, kind="ExternalInput")
    sel_d = nc.dram_tensor("sel", [C, NV], F16, kind="ExternalInput")      # [c,h]=1 if c%8==h
    sel2_d = nc.dram_tensor("sel2", [NV, C], F16, kind="ExternalInput")    # [h,c]=1 if c%8==h
    sel3_d = nc.dram_tensor("sel3", [C, DV], F16, kind="ExternalInput")    # [c,dv]=1/8 if c//8==dv
    ident_d = nc.dram_tensor("ident", [128, 128], F16, kind="ExternalInput")

    # ---- dram outputs ----
    q_d = nc.dram_tensor("q_part", [NB, ACT, 1], F32, kind="ExternalOutput")
    att_d = nc.dram_tensor("att_part", [NB, 2, A, NV, N], F32, kind="ExternalOutput")

    with TileContext(nc) as tc:
        import contextlib
        ctx = contextlib.ExitStack()
        with ctx:
            konst = ctx.enter_context(tc.tile_pool(name="konst", bufs=1))
            adjT_p = ctx.enter_context(tc.tile_pool(name="adjT", bufs=2))
            big_p = ctx.enter_context(tc.tile_pool(name="big", bufs=2))      # nrT
            nh_p = ctx.enter_context(tc.tile_pool(name="nh", bufs=1))
            med_p = ctx.enter_context(tc.tile_pool(name="med", bufs=1))      # ahT/xT/x1T/attT...
            smx_p = ctx.enter_context(tc.tile_pool(name="smx", bufs=1))      # per-n scores f32
            attn_p = ctx.enter_context(tc.tile_pool(name="attn", bufs=1))
            attT_p = ctx.enter_context(tc.tile_pool(name="attT", bufs=2))
            prod_p = ctx.enter_context(tc.tile_pool(name="prod", bufs=3))    # P16/W16
            tiny_p = ctx.enter_context(tc.tile_pool(name="tiny", bufs=4))

            ps_tp = ctx.enter_context(tc.tile_pool(name="ps_tp", bufs=2, space="PSUM"))
            ps_mm = ctx.enter_context(tc.tile_pool(name="ps_mm", bufs=2, space="PSUM"))
            ps_sm = ctx.enter_context(tc.tile_pool(name="ps_sm", bufs=2, space="PSUM"))
            ps_ac = ctx.enter_context(tc.tile_pool(name="ps_ac", bufs=2, space="PSUM"))

            # ---- load constants ----
            def kload(dram, shp, dt):
                t = konst.tile(shp, dt, tag=dram.name)
                nc.sync.dma_start(out=t, in_=dram[:, :])
                return t

            w1 = kload(w1_d, [D, H1], F16)
            w2 = kload(w2_d, [H1, D], F16)
            b1 = kload(b1_d, [H1, 1], F32)
            b2 = kload(b2_d, [D, 1], F32)
            finw = kload(finw_d, [DOUT, ACT], F16)
            finb = kload(finb_d, [ACT, 1], F32)
            sel = kload(sel_d, [C, NV], F16)
            sel2 = kload(sel2_d, [NV, C], F16)
            sel3 = kload(sel3_d, [C, DV], F16)
            ident = kload(ident_d, [128, 128], F16)
            lws = {}
            for l in range(2):
                lws[(l, "ah_w")] = kload(lw[(l, "ah_w")], [D, C], F16)
                lws[(l, "nr_w")] = kload(lw[(l, "nr_w")], [D, C], F16)
                lws[(l, "nh_w")] = kload(lw[(l, "nh_w")], [D, C], F16)
                lws[(l, "o_w")] = kload(lw[(l, "o_w")], [DV, DOUT], F16)
                lws[(l, "ah_b")] = kload(lw[(l, "ah_b")], [C, 1], F32)
                lws[(l, "nr_b")] = kload(lw[(l, "nr_b")], [C, 1], F32)
                lws[(l, "nh_b")] = kload(lw[(l, "nh_b")], [C, 1], F32)
                lws[(l, "o_b")] = kload(lw[(l, "o_b")], [DOUT, 1], F32)

            def cp(out, in_):
                nc.vector.tensor_copy(out, in_)

            def transpose_to(sb_out, in_ap, pdim, fdim, dt=F16):
                """PE-transpose in_ap [pdim, fdim] -> psum [fdim, pdim] -> copy to sb_out."""
                pst = ps_tp.tile([128, 128], dt, tag="tp")
                nc.tensor.transpose(pst[:fdim, :pdim], in_ap, ident[:pdim, :pdim])
                cp(sb_out, pst[:fdim, :pdim])

            def mk_xT(src_T16, dst_nat):
                """xT [32, A] fp16 -> x_nat [128, AT, 32] fp16 via 7 PE transposes."""
                for t in range(AT):
                    a0, sz = t * 128, asize(t)
                    transpose_to(dst_nat[:sz, t, :], src_T16[:, a0:a0 + sz], D, sz)

            # ================= per-batch program =================
            def adj_stream(b, adjT16):
                # adjT_d is host-pre-transposed: [b, a, j=(n,i)].  DMA-cast
                # f32->fp16 straight into the matmul layout, J-chunked so nbr
                # matmuls can start before the whole batch lands.
                for (j0, jch) in JDMA_CHUNKS:
                    nc.sync.dma_start(
                        out=adjT16[:, 0:AT - 1, j0:j0 + jch],
                        in_=adjT_d[b, 0:(AT - 1) * 128, j0:j0 + jch].rearrange(
                            "(t p) j -> p t j", p=128))
                    nc.sync.dma_start(
                        out=adjT16[:ATAIL, AT - 1, j0:j0 + jch],
                        in_=adjT_d[b, (AT - 1) * 128:A, j0:j0 + jch])

            def mlp(b, xT16, xnat16):
                agentT16 = med_p.tile([D, A], F16, tag="agentT")
                nc.sync.dma_start(out=agentT16, in_=agentT_d[b, :, :])
                x1h = med_p.tile([H1, A], F16, tag="x1h")
                for (a0, ch) in A_CHUNKS:
                    pm = ps_mm.tile([H1, 512], F32, tag="mm")
                    nc.tensor.matmul(pm[:, :ch], w1, agentT16[:, a0:a0 + ch],
                                     start=True, stop=True)
                    nc.scalar.activation(x1h[:, a0:a0 + ch], pm[:, :ch], AF.Relu,
                                         bias=b1, scale=1.0)
                for (a0, ch) in A_CHUNKS:
                    pm = ps_mm.tile([D, 512], F32, tag="mm")
                    nc.tensor.matmul(pm[:, :ch], w2, x1h[:, a0:a0 + ch],
                                     start=True, stop=True)
                    nc.scalar.activation(xT16[:, a0:a0 + ch], pm[:, :ch], AF.Relu,
                                         bias=b2, scale=1.0)
                mk_xT(xT16, xnat16)

            def layer(b, l, adjT16, xT16, xnat16, xT16_next, xnat16_next):
                W = lambda nm: lws[(l, nm)]
                # --- ah projection ---
                ahT = med_p.tile([C, A], F16, tag="ahT")
                for (a0, ch) in A_CHUNKS:
                    pm = ps_mm.tile([C, 512], F32, tag="mm")
                    nc.tensor.matmul(pm[:, :ch], W("ah_w"), xT16[:, a0:a0 + ch],
                                     start=True, stop=True)
                    nc.scalar.activation(ahT[:, a0:a0 + ch], pm[:, :ch], AF.Relu,
                                         bias=W("ah_b"), scale=1.0)
                # --- nbr + nr (+ nh), chunk-local nbrT ---
                nrT = big_p.tile([C, J], F16, tag="nrT")
                if l == 0:
                    nhT = nh_p.tile([C, J], F16, tag="nhT")
                else:
                    nbr0 = tiny_p.tile([D, N], F16, tag="nbr0", name=f"nbr0_{b}")
                for (j0, ch) in J_CHUNKS:
                    pn = ps_ac.tile([D, 512], F32, tag="ac")
                    for t in range(AT):
                        sz = asize(t)
                        nc.tensor.matmul(pn[:, :ch], xnat16[:sz, t, :],
                                         adjT16[:sz, t, j0:j0 + ch],
                                         start=(t == 0), stop=(t == AT - 1))
                    nbrTc = prod_p.tile([D, 512], F16, tag="nbrc", name=f"nbrc_{b}_{l}_{j0}")
                    cp(nbrTc[:, :ch], pn[:, :ch])
                    if l == 1:
                        for n in range(N):
                            if j0 <= n * A < j0 + ch:
                                cp(nbr0[:, n:n + 1], nbrTc[:, n * A - j0:n * A - j0 + 1])
                    pm = ps_mm.tile([C, 512], F32, tag="mm")
                    nc.tensor.matmul(pm[:, :ch], W("nr_w"), nbrTc[:, :ch],
                                     start=True, stop=True)
                    nc.scalar.activation(nrT[:, j0:j0 + ch], pm[:, :ch], AF.Relu,
                                         bias=W("nr_b"), scale=1.0)
                    if l == 0:
                        pm2 = ps_mm.tile([C, 512], F32, tag="mm")
                        nc.tensor.matmul(pm2[:, :ch], W("nh_w"), nbrTc[:, :ch],
                                         start=True, stop=True)
                        nc.scalar.activation(nhT[:, j0:j0 + ch], pm2[:, :ch], AF.Relu,
                                             bias=W("nh_b"), scale=1.0)
                # --- scores + softmax (per-n tiles; softmax over agents = free dim) ---
                scores = [smx_p.tile([NV, A], F32, tag=f"sc_{n}", name=f"sc{l}_{n}") for n in range(N)]
                att16 = [attn_p.tile([NV, A], F16, tag=f"att16_{n}", name=f"att{l}_{n}") for n in range(N)]
                negmax = tiny_p.tile([NV, N], F32, tag="negmax")
                sums = tiny_p.tile([NV, N], F32, tag="sums")
                rsum = tiny_p.tile([NV, N], F32, tag="rsum")
                for n in range(N):
                    for (a0, ch) in A_CHUNKS:
                        p16 = prod_p.tile([C, 512], F16, tag="P16")
                        nc.vector.tensor_mul(p16[:, :ch], ahT[:, a0:a0 + ch],
                                             nrT[:, n * A + a0:n * A + a0 + ch])
                        psc = ps_sm.tile([NV, 512], F32, tag="sm")
                        nc.tensor.matmul(psc[:, :ch], sel, p16[:, :ch],
                                         start=True, stop=True)
                        nc.scalar.activation(scores[n][:, a0:a0 + ch],
                                             psc[:, :ch], AF.Copy)
                    nc.vector.reduce_max(out=negmax[:, n:n + 1], in_=scores[n],
                                         axis=AX.X, negate=True)
                    nc.scalar.activation(att16[n], scores[n], AF.Exp,
                                         bias=negmax[:, n:n + 1], scale=1.0,
                                         accum_out=sums[:, n:n + 1])
                nc.vector.reciprocal(rsum, sums)
                for n in range(N):
                    nc.vector.tensor_scalar_mul(att16[n], att16[n], rsum[:, n:n + 1])
                # --- att_record output: PE-transpose to [a, (h,n)] then contiguous DMA ---
                attT = attT_p.tile([128, AT, NV, N], F16, tag="attT", name=f"attT_{b}_{l}")
                for t in range(AT):
                    a0, sz = t * 128, asize(t)
                    for n in range(N):
                        pst = ps_tp.tile([128, 128], F16, tag="tp")
                        nc.tensor.transpose(pst[:sz, :NV], att16[n][:, a0:a0 + sz],
                                            ident[:NV, :NV])
                        cp(attT[:sz, t, :, n], pst[:sz, :NV])
                nc.gpsimd.dma_start(
                    out=att_d[b, l, 0:(AT - 1) * 128, :, :].rearrange(
                        "(t p) h n -> p t h n", p=128),
                    in_=attT[:, 0:AT - 1, :, :])
                nc.gpsimd.dma_start(out=att_d[b, l, (AT - 1) * 128:A, :, :],
                                  in_=attT[:ATAIL, AT - 1, :, :])
                # --- value path ---
                if l == 0:
                    outT = med_p.tile([DV, A], F16, tag="outT")
                    for (a0, ch) in A_CHUNKS:
                        po = ps_ac.tile([DV, 512], F32, tag="ac")
                        for n in range(N):
                            pb = ps_sm.tile([C, 512], F32, tag="sm")
                            nc.tensor.matmul(pb[:, :ch], sel2, att16[n][:, a0:a0 + ch],
                                             start=True, stop=True)
                            w16 = prod_p.tile([C, 512], F16, tag="W16")
                            nc.vector.tensor_mul(w16[:, :ch], pb[:, :ch],
                                                 nhT[:, n * A + a0:n * A + a0 + ch])
                            nc.tensor.matmul(po[:, :ch], sel3, w16[:, :ch],
                                             start=(n == 0), stop=(n == N - 1))
                        cp(outT[:, a0:a0 + ch], po[:, :ch])
                    for (a0, ch) in A_CHUNKS:
                        pm = ps_mm.tile([DOUT, 512], F32, tag="mm")
                        nc.tensor.matmul(pm[:, :ch], W("o_w"), outT[:, a0:a0 + ch],
                                         start=True, stop=True)
                        nc.scalar.activation(xT16_next[:, a0:a0 + ch], pm[:, :ch],
                                             AF.Relu, bias=W("o_b"), scale=1.0)
                    mk_xT(xT16_next, xnat16_next)
                else:
                    # agent-0-only value path feeding q
                    nh0 = tiny_p.tile([C, N], F16, tag="nh0")
                    pm = ps_mm.tile([C, N], F32, tag="mm")
                    nc.tensor.matmul(pm, W("nh_w"), nbr0,
                                     start=True, stop=True)
                    nc.scalar.activation(nh0, pm, AF.Relu, bias=W("nh_b"), scale=1.0)
                    pb = ps_sm.tile([C, N], F32, tag="sm")
                    for n in range(N):
                        nc.tensor.matmul(pb[:, n:n + 1], sel2, att16[n][:, 0:1],
                                         start=(n == 0), stop=(n == N - 1),
                                         skip_group_check=True)
                    wq = tiny_p.tile([C, N], F16, tag="wq")
                    nc.vector.tensor_mul(wq, pb, nh0)
                    wq1 = tiny_p.tile([C, 1], F16, tag="wq1")
                    with nc.allow_low_precision(reason="5-element fp16 sum for q path"):
                        nc.vector.reduce_sum(out=wq1, in_=wq, axis=AX.X)
                    po = ps_ac.tile([DV, 1], F32, tag="ac")
                    nc.tensor.matmul(po, sel3, wq1, start=True, stop=True)
                    out0 = tiny_p.tile([DV, 1], F16, tag="out0")
                    cp(out0, po)
                    px = ps_mm.tile([DOUT, 1], F32, tag="mm")
                    nc.tensor.matmul(px, W("o_w"), out0, start=True, stop=True)
                    x2c = tiny_p.tile([DOUT, 1], F16, tag="x2c")
                    nc.scalar.activation(x2c, px, AF.Relu, bias=W("o_b"), scale=1.0)
                    pq = ps_sm.tile([ACT, 1], F32, tag="sm")
                    nc.tensor.matmul(pq, finw, x2c, start=True, stop=True)
                    qsb = tiny_p.tile([ACT, 1], F32, tag="qsb")
                    nc.vector.tensor_scalar_add(qsb, pq, finb)
                    nc.sync.dma_start(out=q_d[b, :, :], in_=qsb)

            # ================= emit program =================
            state = []
            for b in range(NB):
                xT16 = med_p.tile([D, A], F16, tag=f"xT0_{b}", name=f"xT0_{b}")
                xnat16 = med_p.tile([128, AT, D], F16, tag=f"xnat0_{b}", name=f"xnat0_{b}")
                mlp(b, xT16, xnat16)
                adjT16 = adjT_p.tile([128, AT, J], F16, tag="adjT", name=f"adjT_{b}")
                adj_stream(b, adjT16)
                state.append((adjT16, xT16, xnat16))
            nxt = []
            for b in range(NB):
                adjT16, xT16, xnat16 = state[b]
                xT16_1 = med_p.tile([D, A], F16, tag=f"xT1_{b}", name=f"xT1_{b}")
                xnat16_1 = med_p.tile([128, AT, D], F16, tag=f"xnat1_{b}", name=f"xnat1_{b}")
                layer(b, 0, adjT16, xT16, xnat16, xT16_1, xnat16_1)
                nxt.append((xT16_1, xnat16_1))
            for b in range(NB):
                adjT16, _, _ = state[b]
                xT16_1, xnat16_1 = nxt[b]
                layer(b, 1, adjT16, xT16_1, xnat16_1, None, None)

    nc.compile()
    return nc


_NC_CACHE = {}


def _get_nc():
    if "nc" not in _NC_CACHE:
        _NC_CACHE["nc"] = build_nc()
    return _NC_CACHE["nc"]


def _selector_consts():
    c = np.arange(C)
    sel = (c[:, None] % NV == np.arange(NV)[None, :]).astype(np.float16)
    sel2 = sel.T.copy()
    sel3 = ((c[:, None] // NV == np.arange(DV)[None, :]).astype(np.float32) / NV
            ).astype(np.float16)
    ident = np.eye(128, dtype=np.float16)
    return sel, sel2, sel3, ident


def make_in_maps(inputs):
    inp = {k: np.asarray(v) for k, v in inputs.items()}
    sel, sel2, sel3, ident = _selector_consts()
    shared = {
        "w1": inp["mlp_w1"].astype(np.float16),
        "w2": inp["mlp_w2"].astype(np.float16),
        "b1": inp["mlp_b1"].reshape(H1, 1).astype(np.float32),
        "b2": inp["mlp_b2"].reshape(D, 1).astype(np.float32),
        "finw": inp["fin_w"].astype(np.float16),
        "finb": inp["fin_b"].reshape(ACT, 1).astype(np.float32),
        "sel": sel, "sel2": sel2, "sel3": sel3, "ident": ident,
    }
    for l in range(2):
        for nm in ("ah_w", "nr_w", "nh_w", "o_w"):
            shared[f"l{l}_{nm}"] = inp[f"l{l}_{nm}"].astype(np.float16)
        for nm in ("ah_b", "nr_b", "nh_b", "o_b"):
            v = inp[f"l{l}_{nm}"]
            shared[f"l{l}_{nm}"] = v.reshape(v.shape[0], 1).astype(np.float32)

    in_maps = []
    for core in range(NCORES):
        m = dict(shared)
        m["agentT"] = np.ascontiguousarray(
            inp["agent"][core * NB:(core + 1) * NB].transpose(0, 2, 1)
        ).astype(np.float16)
        # adjT[b, a, (n, i)] = adj[b, i, n, a]  (n-major j so per-n slices are
        # contiguous in the free dim on-chip)
        m["adjT"] = np.ascontiguousarray(
            inp["adj"][core * NB:(core + 1) * NB].transpose(0, 3, 2, 1)
        ).reshape(NB, A, J).astype(np.float16)
        in_maps.append(m)
    return in_maps


def kernel(**inputs):
    in_maps = make_in_maps(inputs)
    nc = _get_nc()
    res = run_bass_kernel_spmd(nc, in_maps, core_ids=list(range(NCORES)))
    q = np.concatenate([r["q_part"][:, :, 0] for r in res.results], axis=0)
    att = np.concatenate([r["att_part"] for r in res.results], axis=0)
    return q.astype(np.float32), att.astype(np.float32)


if __name__ == "__main__":
    rng = np.random.default_rng(0)
    # smoke test with random weights
    fake = {
        "agent": rng.standard_normal((B, A, D), dtype=np.float32),
        "adj": rng.random((B, A, N, A), dtype=np.float32),
        "mlp_w1": rng.standard_normal((D, H1), dtype=np.float32) * 0.05,
        "mlp_b1": rng.standard_normal((H1,), dtype=np.float32) * 0.05,
        "mlp_w2": rng.standard_normal((H1, D), dtype=np.float32) * 0.05,
        "mlp_b2": rng.standard_normal((D,), dtype=np.float32) * 0.05,
        "fin_w": rng.standard_normal((DOUT, ACT), dtype=np.float32) * 0.05,
        "fin_b": rng.standard_normal((ACT,), dtype=np.float32) * 0.05,
    }
    for l in range(2):
        for nm, shp in (("ah_w", (D, C)), ("ah_b", (C,)), ("nr_w", (D, C)),
                        ("nr_b", (C,)), ("nh_w", (D, C)), ("nh_b", (C,)),
                        ("o_w", (DV, DOUT)), ("o_b", (DOUT,))):
            fake[f"l{l}_{nm}"] = rng.standard_normal(shp).astype(np.float32) * 0.05
    q, att = kernel(**fake)
    print("q", q.shape, q.dtype, "att", att.shape, att.dtype)


# revision 17
# speedup vs baseline: 1.1543x; 1.1543x over previous
"""Colight GNN message-passing kernel for 8x TRN2 NeuronCores.

Sharding: data-parallel over batch (16 batches -> 2 per core), weights replicated.
Per core, per batch:
  - MLP: x = relu(relu(agent@W1+b1)@W2+b2), computed transposed (xT [32, A]).
  - adj [A,N,A] is DMA'd once with inline f32->fp16 cast, PE-transposed into
    adjT [a, (n,i)] fp16 cached in SBUF, reused by both attention layers.
  - nbrT = (adj @ x)^T via matmul(lhsT=x_nat, rhs=adjT) accumulating over a-tiles.
  - Projections nr/nh/ah via stationary weights (lhsT=w [32,128], rhs=activationT).
  - scores via DVE product P = ahT*nrT + selector matmul (heads interleaved c=dv*8+h).
  - softmax over agents = free-dim softmax on scores40 [40(n,h), A].
  - out einsum via attB (selector broadcast) * nhT, accumulated over n in PSUM.
  - Layer 1 value path only computed for agent 0 (only agent 0 feeds q).
"""
import sys
sys.path.insert(0, '/opt/trn_rl_repo')

import numpy as np

import concourse.bass as bass
import concourse.mybir as mybir
from concourse import bacc
from concourse.tile import TileContext
from concourse.bass_utils import run_bass_kernel_spmd

# dims
B, A, N = 16, 859, 5
D = 32
H1 = 128
DV, NV, DOUT, ACT = 16, 8, 32, 8
C = DV * NV                    # 128, c = dv*8 + h
NB = 2                         # batches per core
NCORES = 8
J = N * A                      # 4295 flattened (n, i)
AT = (A + 127) // 128          # 7 a-tiles
ATAIL = A - (AT - 1) * 128     # 91
F16 = mybir.dt.float16
F32 = mybir.dt.float32
AF = mybir.ActivationFunctionType
AX = mybir.AxisListType


def _chunks(total, size):
    out = []
    o = 0
    while o < total:
        out.append((o, min(size, total - o)))
        o += size
    return out


J_CHUNKS = _chunks(J, 512)          # 9 chunks for nbr/nr/nh over (n,i)
A_CHUNKS = _chunks(A, 512)          # 2 chunks over agents
JDMA_CHUNKS = [(0, 512)] + [(512 + o, c) for (o, c) in _chunks(J - 512, 1024)]



# packed-constant blobs: (name, partitions, free)
KONST16_SPECS = ([("w1", D, H1), ("w2", H1, D), ("finw", DOUT, ACT),
                  ("sel", C, NV), ("sel2", NV, C), ("sel3", C, DV),
                  ("ident", 128, 128)] +
                 [(f"l{l}_{nm}", D if nm != "o_w" else DV,
                   C if nm != "o_w" else DOUT)
                  for l in range(2) for nm in ("ah_w", "nr_w", "nh_w", "o_w")])
KONST32_SPECS = ([("b1", H1, 1), ("b2", D, 1), ("finb", ACT, 1)] +
                 [(f"l{l}_{nm}", C if nm != "o_b" else DOUT, 1)
                  for l in range(2) for nm in ("ah_b", "nr_b", "nh_b", "o_b")])


def _konst_offsets(specs):
    offs, o = {}, 0
    for name, p, f in specs:
        offs[name] = (o, p, f)
        o += f
    return offs, o


K16_OFFS, K16_TOT = _konst_offsets(KONST16_SPECS)
K32_OFFS, K32_TOT = _konst_offsets(KONST32_SPECS)

def asize(t):
    return 128 if t < AT - 1 else ATAIL


def build_nc():
    nc = bacc.Bacc("TRN2", target_bir_lowering=False)

    # ---- dram inputs ----
    agentT_d = nc.dram_tensor("agentT", [NB, D, A], F16, kind="ExternalInput")
    adjT_d = nc.dram_tensor("adjT", [NB, A, J], F16, kind="ExternalInput")
    kb16_d = nc.dram_tensor("kblob16", [128, K16_TOT], F16, kind="ExternalInput")
    kb32_d = nc.dram_tensor("kblob32", [128, K32_TOT], F32, kind="ExternalInput")

    # ---- dram outputs ----
    q_d = nc.dram_tensor("q_part", [NB, ACT, 1], F32, kind="ExternalOutput")
    att_d = nc.dram_tensor("att_part", [NB, 2, A, NV, N], F32, kind="ExternalOutput")

    with TileContext(nc) as tc:
        import contextlib
        ctx = contextlib.ExitStack()
        with ctx:
            konst = ctx.enter_context(tc.tile_pool(name="konst", bufs=1))
            adjT_p = ctx.enter_context(tc.tile_pool(name="adjT", bufs=2))
            big_p = ctx.enter_context(tc.tile_pool(name="big", bufs=2))      # nrT
            nh_p = ctx.enter_context(tc.tile_pool(name="nh", bufs=1))
            med_p = ctx.enter_context(tc.tile_pool(name="med", bufs=1))      # ahT/xT/x1T/attT...
            smx_p = ctx.enter_context(tc.tile_pool(name="smx", bufs=1))      # per-n scores f32
            attn_p = ctx.enter_context(tc.tile_pool(name="attn", bufs=1))
            attT_p = ctx.enter_context(tc.tile_pool(name="attT", bufs=2))
            prod_p = ctx.enter_context(tc.tile_pool(name="prod", bufs=3))    # P16/W16
            tiny_p = ctx.enter_context(tc.tile_pool(name="tiny", bufs=4))

            ps_tp = ctx.enter_context(tc.tile_pool(name="ps_tp", bufs=1, space="PSUM"))
            ps_mm = ctx.enter_context(tc.tile_pool(name="ps_mm", bufs=2, space="PSUM"))
            ps_sm = ctx.enter_context(tc.tile_pool(name="ps_sm", bufs=2, space="PSUM"))
            ps_nb = ctx.enter_context(tc.tile_pool(name="ps_nb", bufs=2, space="PSUM"))
            ps_ou = ctx.enter_context(tc.tile_pool(name="ps_ou", bufs=1, space="PSUM"))

            # ---- load constants: 2 packed blob DMAs ----
            kb16 = konst.tile([128, K16_TOT], F16, tag="kb16")
            nc.sync.dma_start(out=kb16, in_=kb16_d[:, :])
            kb32 = konst.tile([128, K32_TOT], F32, tag="kb32")
            nc.sync.dma_start(out=kb32, in_=kb32_d[:, :])

            def kv16(name):
                o, p, f = K16_OFFS[name]
                return kb16[:p, o:o + f]

            def kv32(name):
                o, p, f = K32_OFFS[name]
                return kb32[:p, o:o + f]

            w1, w2, finw = kv16("w1"), kv16("w2"), kv16("finw")
            sel, sel2, sel3, ident = kv16("sel"), kv16("sel2"), kv16("sel3"), kv16("ident")
            b1, b2, finb = kv32("b1"), kv32("b2"), kv32("finb")
            lws = {}
            for l in range(2):
                for nm in ("ah_w", "nr_w", "nh_w", "o_w"):
                    lws[(l, nm)] = kv16(f"l{l}_{nm}")
                for nm in ("ah_b", "nr_b", "nh_b", "o_b"):
                    lws[(l, nm)] = kv32(f"l{l}_{nm}")

            def cp(out, in_):
                nc.vector.tensor_copy(out, in_)

            def transpose_to(sb_out, in_ap, pdim, fdim, dt=F16):
                """PE-transpose in_ap [pdim, fdim] -> psum [fdim, pdim] -> copy to sb_out."""
                pst = ps_tp.tile([128, 128], dt, tag="tp")
                nc.tensor.transpose(pst[:fdim, :pdim], in_ap, ident[:pdim, :pdim])
                cp(sb_out, pst[:fdim, :pdim])

            def mk_xT(src_T16, dst_nat):
                """xT [32, A] fp16 -> x_nat [128, AT, 32] fp16 via 7 PE transposes."""
                for t in range(AT):
                    a0, sz = t * 128, asize(t)
                    transpose_to(dst_nat[:sz, t, :], src_T16[:, a0:a0 + sz], D, sz)

            # ================= per-batch program =================
            def adj_chunk(b, adjT16, j0, jch):
                # adjT_d is host-pre-transposed fp16: [b, a, j=(n,i)] -> direct
                # HWDGE DMA into the matmul layout, chunked over j.
                nc.sync.dma_start(
                    out=adjT16[:, 0:AT - 1, j0:j0 + jch],
                    in_=adjT_d[b, 0:(AT - 1) * 128, j0:j0 + jch].rearrange(
                        "(t p) j -> p t j", p=128))
                nc.sync.dma_start(
                    out=adjT16[:ATAIL, AT - 1, j0:j0 + jch],
                    in_=adjT_d[b, (AT - 1) * 128:A, j0:j0 + jch])

            def mlp(b, xT16, xnat16):
                agentT16 = med_p.tile([D, A], F16, tag="agentT")
                nc.sync.dma_start(out=agentT16, in_=agentT_d[b, :, :])
                x1h = med_p.tile([H1, A], F16, tag="x1h")
                for (a0, ch) in A_CHUNKS:
                    pm = ps_mm.tile([H1, 512], F32, tag="mm")
                    nc.tensor.matmul(pm[:, :ch], w1, agentT16[:, a0:a0 + ch],
                                     start=True, stop=True)
                    nc.scalar.activation(x1h[:, a0:a0 + ch], pm[:, :ch], AF.Relu,
                                         bias=b1, scale=1.0)
                for (a0, ch) in A_CHUNKS:
                    pm = ps_mm.tile([D, 512], F32, tag="mm")
                    nc.tensor.matmul(pm[:, :ch], w2, x1h[:, a0:a0 + ch],
                                     start=True, stop=True)
                    nc.scalar.activation(xT16[:, a0:a0 + ch], pm[:, :ch], AF.Relu,
                                         bias=b2, scale=1.0)
                mk_xT(xT16, xnat16)

            def layer(b, l, adjT16, xT16, xnat16, xT16_next, xnat16_next):
                W = lambda nm: lws[(l, nm)]
                # --- ah projection ---
                ahT = med_p.tile([C, A], F16, tag="ahT")
                for (a0, ch) in A_CHUNKS:
                    pm = ps_mm.tile([C, 512], F32, tag="mm")
                    nc.tensor.matmul(pm[:, :ch], W("ah_w"), xT16[:, a0:a0 + ch],
                                     start=True, stop=True)
                    nc.scalar.activation(ahT[:, a0:a0 + ch], pm[:, :ch], AF.Relu,
                                         bias=W("ah_b"), scale=1.0)
                # --- nbr + nr (+ nh), chunk-local nbrT ---
                nrT = big_p.tile([C, J], F16, tag="nrT")
                if l == 0:
                    nhT = nh_p.tile([C, J], F16, tag="nhT")
                else:
                    nbr0 = tiny_p.tile([D, N], F16, tag="nbr0", name=f"nbr0_{b}")
                for (j0, ch) in J_CHUNKS:
                    pn = ps_nb.tile([D, 512], F32, tag="nb")
                    for t in range(AT):
                        sz = asize(t)
                        nc.tensor.matmul(pn[:, :ch], xnat16[:sz, t, :],
                                         adjT16[:sz, t, j0:j0 + ch],
                                         start=(t == 0), stop=(t == AT - 1))
                    nbrTc = prod_p.tile([D, 512], F16, tag="nbrc", name=f"nbrc_{b}_{l}_{j0}")
                    cp(nbrTc[:, :ch], pn[:, :ch])
                    if l == 1:
                        for n in range(N):
                            if j0 <= n * A < j0 + ch:
                                cp(nbr0[:, n:n + 1], nbrTc[:, n * A - j0:n * A - j0 + 1])
                    pm = ps_mm.tile([C, 512], F32, tag="mm")
                    nc.tensor.matmul(pm[:, :ch], W("nr_w"), nbrTc[:, :ch],
                                     start=True, stop=True)
                    nc.scalar.activation(nrT[:, j0:j0 + ch], pm[:, :ch], AF.Relu,
                                         bias=W("nr_b"), scale=1.0)
                    if l == 0:
                        pm2 = ps_mm.tile([C, 512], F32, tag="mm")
                        nc.tensor.matmul(pm2[:, :ch], W("nh_w"), nbrTc[:, :ch],
                                         start=True, stop=True)
                        nc.scalar.activation(nhT[:, j0:j0 + ch], pm2[:, :ch], AF.Relu,
                                             bias=W("nh_b"), scale=1.0)
                # --- scores + softmax (per-n tiles; softmax over agents = free dim) ---
                scores = [smx_p.tile([NV, A], F32, tag=f"sc_{n}", name=f"sc{l}_{n}") for n in range(N)]
                att16 = [attn_p.tile([NV, A], F16, tag=f"att16_{n}", name=f"att{l}_{n}") for n in range(N)]
                negmax = tiny_p.tile([NV, N], F32, tag="negmax")
                sums = tiny_p.tile([NV, N], F32, tag="sums")
                rsum = tiny_p.tile([NV, N], F32, tag="rsum")
                for n in range(N):
                    for (a0, ch) in A_CHUNKS:
                        p16 = prod_p.tile([C, 512], F16, tag="P16")
                        nc.vector.tensor_mul(p16[:, :ch], ahT[:, a0:a0 + ch],
                                             nrT[:, n * A + a0:n * A + a0 + ch])
                        psc = ps_sm.tile([NV, 512], F32, tag="sm")
                        nc.tensor.matmul(psc[:, :ch], sel, p16[:, :ch],
                                         start=True, stop=True)
                        nc.scalar.activation(scores[n][:, a0:a0 + ch],
                                             psc[:, :ch], AF.Copy)
                    nc.vector.reduce_max(out=negmax[:, n:n + 1], in_=scores[n],
                                         axis=AX.X, negate=True)
                    nc.scalar.activation(att16[n], scores[n], AF.Exp,
                                         bias=negmax[:, n:n + 1], scale=1.0,
                                         accum_out=sums[:, n:n + 1])
                nc.vector.reciprocal(rsum, sums)
                for n in range(N):
                    nc.vector.tensor_scalar_mul(att16[n], att16[n], rsum[:, n:n + 1])
                # --- att_record output: PE-transpose to [a, (h,n)] then contiguous DMA ---
                attT = attT_p.tile([128, AT, NV, N], F16, tag="attT", name=f"attT_{b}_{l}")
                for t in range(AT):
                    a0, sz = t * 128, asize(t)
                    for n in range(N):
                        pst = ps_tp.tile([128, 128], F16, tag="tp")
                        nc.tensor.transpose(pst[:sz, :NV], att16[n][:, a0:a0 + sz],
                                            ident[:NV, :NV])
                        cp(attT[:sz, t, :, n], pst[:sz, :NV])
                nc.gpsimd.dma_start(
                    out=att_d[b, l, 0:(AT - 1) * 128, :, :].rearrange(
                        "(t p) h n -> p t h n", p=128),
                    in_=attT[:, 0:AT - 1, :, :])
                nc.gpsimd.dma_start(out=att_d[b, l, (AT - 1) * 128:A, :, :],
                                  in_=attT[:ATAIL, AT - 1, :, :])
                # --- value path ---
                if l == 0:
                    outT = med_p.tile([DV, A], F16, tag="outT")
                    for (a0, ch) in A_CHUNKS:
                        po = ps_ou.tile([DV, 512], F32, tag="ou")
                        for n in range(N):
                            pb = ps_sm.tile([C, 512], F32, tag="sm")
                            nc.tensor.matmul(pb[:, :ch], sel2, att16[n][:, a0:a0 + ch],
                                             start=True, stop=True)
                            w16 = prod_p.tile([C, 512], F16, tag="W16")
                            nc.vector.tensor_mul(w16[:, :ch], pb[:, :ch],
                                                 nhT[:, n * A + a0:n * A + a0 + ch])
                            nc.tensor.matmul(po[:, :ch], sel3, w16[:, :ch],
                                             start=(n == 0), stop=(n == N - 1))
                        cp(outT[:, a0:a0 + ch], po[:, :ch])
                    for (a0, ch) in A_CHUNKS:
                        pm = ps_mm.tile([DOUT, 512], F32, tag="mm")
                        nc.tensor.matmul(pm[:, :ch], W("o_w"), outT[:, a0:a0 + ch],
                                         start=True, stop=True)
                        nc.scalar.activation(xT16_next[:, a0:a0 + ch], pm[:, :ch],
                                             AF.Relu, bias=W("o_b"), scale=1.0)
                    mk_xT(xT16_next, xnat16_next)
                else:
                    # agent-0-only value path feeding q
                    nh0 = tiny_p.tile([C, N], F16, tag="nh0")
                    pm = ps_mm.tile([C, N], F32, tag="mm")
                    nc.tensor.matmul(pm, W("nh_w"), nbr0,
                                     start=True, stop=True)
                    nc.scalar.activation(nh0, pm, AF.Relu, bias=W("nh_b"), scale=1.0)
                    pb = ps_sm.tile([C, N], F32, tag="sm")
                    for n in range(N):
                        nc.tensor.matmul(pb[:, n:n + 1], sel2, att16[n][:, 0:1],
                                         start=(n == 0), stop=(n == N - 1),
                                         skip_group_check=True)
                    wq = tiny_p.tile([C, N], F16, tag="wq")
                    nc.vector.tensor_mul(wq, pb, nh0)
                    wq1 = tiny_p.tile([C, 1], F16, tag="wq1")
                    with nc.allow_low_precision(reason="5-element fp16 sum for q path"):
                        nc.vector.reduce_sum(out=wq1, in_=wq, axis=AX.X)
                    po = ps_ou.tile([DV, 1], F32, tag="ou")
                    nc.tensor.matmul(po, sel3, wq1, start=True, stop=True)
                    out0 = tiny_p.tile([DV, 1], F16, tag="out0")
                    cp(out0, po)
                    px = ps_mm.tile([DOUT, 1], F32, tag="mm")
                    nc.tensor.matmul(px, W("o_w"), out0, start=True, stop=True)
                    x2c = tiny_p.tile([DOUT, 1], F16, tag="x2c")
                    nc.scalar.activation(x2c, px, AF.Relu, bias=W("o_b"), scale=1.0)
                    pq = ps_sm.tile([ACT, 1], F32, tag="sm")
                    nc.tensor.matmul(pq, finw, x2c, start=True, stop=True)
                    qsb = tiny_p.tile([ACT, 1], F32, tag="qsb")
                    nc.vector.tensor_scalar_add(qsb, pq, finb)
                    nc.sync.dma_start(out=q_d[b, :, :], in_=qsb)

            # ================= emit program =================
            state = []
            for b in range(NB):
                xT16 = med_p.tile([D, A], F16, tag=f"xT0_{b}", name=f"xT0_{b}")
                xnat16 = med_p.tile([128, AT, D], F16, tag=f"xnat0_{b}", name=f"xnat0_{b}")
                mlp(b, xT16, xnat16)
                adjT16 = adjT_p.tile([128, AT, J], F16, tag="adjT", name=f"adjT_{b}")
                state.append((adjT16, xT16, xnat16))
            for (j0, jch) in JDMA_CHUNKS:
                for b in range(NB):
                    adj_chunk(b, state[b][0], j0, jch)
            nxt = []
            for b in range(NB):
                adjT16, xT16, xnat16 = state[b]
                xT16_1 = med_p.tile([D, A], F16, tag=f"xT1_{b}", name=f"xT1_{b}")
                xnat16_1 = med_p.tile([128, AT, D], F16, tag=f"xnat1_{b}", name=f"xnat1_{b}")
                layer(b, 0, adjT16, xT16, xnat16, xT16_1, xnat16_1)
                nxt.append((xT16_1, xnat16_1))
            for b in range(NB):
                adjT16, _, _ = state[b]
                xT16_1, xnat16_1 = nxt[b]
                layer(b, 1, adjT16, xT16_1, xnat16_1, None, None)

    nc.compile()
    return nc


_NC_CACHE = {}


def _get_nc():
    if "nc" not in _NC_CACHE:
        _NC_CACHE["nc"] = build_nc()
    return _NC_CACHE["nc"]


def _selector_consts():
    c = np.arange(C)
    sel = (c[:, None] % NV == np.arange(NV)[None, :]).astype(np.float16)
    sel2 = sel.T.copy()
    sel3 = ((c[:, None] // NV == np.arange(DV)[None, :]).astype(np.float32) / NV
            ).astype(np.float16)
    ident = np.eye(128, dtype=np.float16)
    return sel, sel2, sel3, ident


def make_in_maps(inputs):
    inp = {k: np.asarray(v) for k, v in inputs.items()}
    sel, sel2, sel3, ident = _selector_consts()
    vals = {
        "w1": inp["mlp_w1"], "w2": inp["mlp_w2"], "finw": inp["fin_w"],
        "sel": sel, "sel2": sel2, "sel3": sel3, "ident": ident,
        "b1": inp["mlp_b1"].reshape(H1, 1), "b2": inp["mlp_b2"].reshape(D, 1),
        "finb": inp["fin_b"].reshape(ACT, 1),
    }
    for l in range(2):
        for nm in ("ah_w", "nr_w", "nh_w", "o_w"):
            vals[f"l{l}_{nm}"] = inp[f"l{l}_{nm}"]
        for nm in ("ah_b", "nr_b", "nh_b", "o_b"):
            v = inp[f"l{l}_{nm}"]
            vals[f"l{l}_{nm}"] = v.reshape(v.shape[0], 1)
    kb16 = np.zeros((128, K16_TOT), np.float16)
    for name, p, f in KONST16_SPECS:
        o = K16_OFFS[name][0]
        kb16[:p, o:o + f] = vals[name].astype(np.float16)
    kb32 = np.zeros((128, K32_TOT), np.float32)
    for name, p, f in KONST32_SPECS:
        o = K32_OFFS[name][0]
        kb32[:p, o:o + f] = vals[name].astype(np.float32)
    shared = {"kblob16": kb16, "kblob32": kb32}

    in_maps = []
    for core in range(NCORES):
        m = dict(shared)
        m["agentT"] = np.ascontiguousarray(
            inp["agent"][core * NB:(core + 1) * NB].transpose(0, 2, 1)
        ).astype(np.float16)
        # adjT[b, a, (n, i)] = adj[b, i, n, a]  (n-major j so per-n slices are
        # contiguous in the free dim on-chip)
        m["adjT"] = np.ascontiguousarray(
            inp["adj"][core * NB:(core + 1) * NB].transpose(0, 3, 2, 1)
        ).reshape(NB, A, J).astype(np.float16)
        in_maps.append(m)
    return in_maps


def kernel(**inputs):
    in_maps = make_in_maps(inputs)
    nc = _get_nc()
    res = run_bass_kernel_spmd(nc, in_maps, core_ids=list(range(NCORES)))
    q = np.concatenate([r["q_part"][:, :, 0] for r in res.results], axis=0)
    att = np.concatenate([r["att_part"] for r in res.results], axis=0)
    return q.astype(np.float32), att.astype(np.float32)


if __name__ == "__main__":
    rng = np.random.default_rng(0)
    # smoke test with random weights
    fake = {
        "agent": rng.standard_normal((B, A, D), dtype=np.float32),
        "adj": rng.random((B, A, N, A), dtype=np.float32),
        "mlp_w1": rng.standard_normal((D, H1), dtype=np.float32) * 0.05,
        "mlp_b1": rng.standard_normal((H1,), dtype=np.float32) * 0.05,
        "mlp_w2": rng.standard_normal((H1, D), dtype=np.float32) * 0.05,
        "mlp_b2": rng.standard_normal((D,), dtype=np.float32) * 0.05,
        "fin_w": rng.standard_normal((DOUT, ACT), dtype=np.float32) * 0.05,
        "fin_b": rng.standard_normal((ACT,), dtype=np.float32) * 0.05,
    }
    for l in range(2):
        for nm, shp in (("ah_w", (D, C)), ("ah_b", (C,)), ("nr_w", (D, C)),
                        ("nr_b", (C,)), ("nh_w", (D, C)), ("nh_b", (C,)),
                        ("o_w", (DV, DOUT)), ("o_b", (DOUT,))):
            fake[f"l{l}_{nm}"] = rng.standard_normal(shp).astype(np.float32) * 0.05
    q, att = kernel(**fake)
    print("q", q.shape, q.dtype, "att", att.shape, att.dtype)
